# revision 18
# baseline (speedup 1.0000x reference)
"""Trainium2 Bass kernel: batched Clifford-algebra geometric product, G(3,0,0).

out[n,k] = sum_{i,j} a[n,i] * b[n,j] * cayley[i,j,k],  a/b: [4194304, 8] f32.

Sharding: pure data parallel over 8 NeuronCores (batch axis split into 8
contiguous shards); the cayley structure is compiled into the instruction
stream, so nothing is replicated at runtime.

Per-core algorithm (plan "v2d", the default): instead of expanding all 64
blade products (120+ flops/row), use the algebra isomorphism
Cl(3,0) ~= M2(C).  Per row:

  ab-pack (host): ab = [a/2, b]  (one 2 MB load DMA per tile, /2 is exact)
  rho-build:  A = rho(a)/2, B = rho(b) -- 4 DVE add/sub ops (each entry of
              the 2x2 complex matrices is a +- pair of blade coords)
  product:    C = A @ B -- 8 DVE muls (4 real products per complex mult,
              quad order [xu, yu, xv, yv]) + 1 add (t-contraction) +
              sub/add (re/im combine)
  rho-inverse: out = rho^-1(C) -- 3 DVE add/sub ops

80 DVE elems/row total vs 192 for the direct expansion; rows live in SBUF
partitions (128 rows in flight, 256 rows/partition/tile), all gathers are
small strided access patterns (<= 3 free dims, the ISA limit).  The layouts
of A/B (re/im interleaved), m (gray-code rc order), and C were chosen so
every gather/scatter in the pipeline is affine.

DVE streaming time ~340 us/core; HBM traffic 48 MB/core ~= 150-250 us --
the kernel pipelines tiles so the two mostly overlap (~200-450 us/core
measured, environment is noisy).
"""

import os
import sys

import numpy as np

for _p in ("/opt/trn_rl_repo",):
    if os.path.isdir(_p) and _p not in sys.path:
        sys.path.insert(0, _p)

import concourse.bass as bass  # noqa: E402
import concourse.mybir as mybir  # noqa: E402
from concourse import bacc, tile  # noqa: E402
from concourse.bass_utils import run_bass_kernel_spmd  # noqa: E402
from contextlib import ExitStack  # noqa: E402

N_FULL = 4194304
N_CORES = 8
P = 128  # SBUF partitions


# ---------------------------------------------------------------------------
# Cayley structure (hardcoded G(3,0,0), short-lex blade order)
# ---------------------------------------------------------------------------
def _build_cayley(metric=(1, 1, 1)):
    n = len(metric)
    nb = 1 << n
    bitmaps = sorted(range(nb), key=lambda b: (bin(b).count("1"), b))
    b2i = {bm: i for i, bm in enumerate(bitmaps)}
    C = np.zeros((nb, nb, nb), np.float32)
    for i, bi in enumerate(bitmaps):
        for j, bj in enumerate(bitmaps):
            acc = bi >> 1
            swaps = 0
            while acc:
                swaps += bin(acc & bj).count("1")
                acc >>= 1
            sign = -1.0 if (swaps & 1) else 1.0
            for t in range(n):
                if (bi & bj) & (1 << t):
                    sign *= metric[t]
            if sign:
                C[i, j, b2i[bi ^ bj]] = sign
    return C, np.array(bitmaps)


def _sign_table():
    """S64[8*kb + ib] = sign of (blade ib) * (blade ib^kb) in bitmap order."""
    C, bm = _build_cayley()
    Q = np.argsort(bm)  # bitmap -> short-lex index
    S = np.zeros(64, np.float32)
    for kb in range(8):
        for ib in range(8):
            S[8 * kb + ib] = C[Q[ib], Q[ib ^ kb], Q[kb]]
    return S


# ---------------------------------------------------------------------------
# Op plan: every entry is one engine instruction.
#   (engine, alu, dst, ddims, doff, src0, dims0, off0, src1, dims1, off1)
# dims are [step, count] pairs over the per-row column space of each buffer;
# a leading [row_step=width, t] dim is added at emit time (width 0 for "S").
# Buffer widths (f32 cols per row): ta/tb/tbp/to: 8, za/z: 64, t1: 32, t2: 16.
# ---------------------------------------------------------------------------
_WIDTH = {"ta": 8, "tb": 8, "tbp": 8, "za": 64, "z": 64, "t1": 32, "t2": 16,
          "to": 8, "S": 64}


def _op_plan():
    """Every AP below has at most 3 free dims (+1 row dim added at emit time
    is merged away by the AP optimizer only when contiguous, so ISA allows
    row + 2 more); the ISA TENSOR3D pattern limit is 3 free dims total, so
    each op's dims list must have <= 2 entries beyond what merges with the
    row dim. We keep every dims list <= 2 entries."""
    ops = []
    # b short-lex -> bitmap order (swap cols 3,4); runs on ScalarE.
    ops.append(("scalar", "copy", "tbp", [[5, 2], [1, 3]], 0,
                "tb", [[5, 2], [1, 3]], 0, None, None, 0))
    ops.append(("scalar", "copy", "tbp", [[1, 2]], 3,
                "tb", [[-1, 2]], 4, None, None, 0))
    # sign-fold: za[:, 8k+ib] = S[8k+ib] * a~[:, ib]; a's 3<->4 swap folded in.
    for off in (0, 5):  # ib chunks {0,1,2} and {5,6,7}
        ops.append(("vector", "mult", "za", [[8, 8], [1, 3]], off,
                    "ta", [[0, 8], [1, 3]], off,
                    "S", [[8, 8], [1, 3]], off))
    ops.append(("vector", "mult", "za", [[8, 8], [1, 2]], 3,
                "ta", [[0, 8], [-1, 2]], 4,
                "S", [[8, 8], [1, 2]], 3))
    # products: z[:, 8k+ib] = za[:, 8k+ib] * tbp[:, ib^k].
    # XOR gather = bit-strided dims; merge adjacent bit dims when signs align.
    for k in range(8):
        s2 = -4 if k & 4 else 4
        s1 = -2 if k & 2 else 2
        s0 = -1 if k & 1 else 1
        if (s1 > 0) == (s0 > 0):      # merge (b1,b0) -> step s0 x4
            ops.append(("vector", "mult", "z", [[4, 2], [1, 4]], 8 * k,
                        "za", [[4, 2], [1, 4]], 8 * k,
                        "tbp", [[s2, 2], [s0, 4]], k))
        elif (s2 > 0) == (s1 > 0):    # merge (b2,b1) -> step s1 x4
            ops.append(("vector", "mult", "z", [[2, 4], [1, 2]], 8 * k,
                        "za", [[2, 4], [1, 2]], 8 * k,
                        "tbp", [[s1, 4], [s0, 2]], k))
        else:                          # k in {2,5}: split over b2
            for i2 in (0, 1):
                ops.append(("vector", "mult",
                            "z", [[2, 2], [1, 2]], 8 * k + 4 * i2,
                            "za", [[2, 2], [1, 2]], 8 * k + 4 * i2,
                            "tbp", [[s1, 2], [s0, 2]], k ^ (4 * i2)))
    # tree adds over ib (4 inner cols at a time)
    ops.append(("vector", "add", "t1", [[4, 8], [1, 4]], 0,
                "z", [[8, 8], [1, 4]], 0, "z", [[8, 8], [1, 4]], 4))
    ops.append(("vector", "add", "t2", [[2, 8], [1, 2]], 0,
                "t1", [[4, 8], [1, 2]], 0, "t1", [[4, 8], [1, 2]], 2))
    # final add, bitmap -> short-lex fold on the output
    ops.append(("vector", "add", "to", [[5, 2], [1, 3]], 0,
                "t2", [[10, 2], [2, 3]], 0, "t2", [[10, 2], [2, 3]], 1))
    ops.append(("vector", "add", "to", [[-1, 2]], 4,
                "t2", [[2, 2]], 6, "t2", [[2, 2]], 7))
    return ops


_WIDTH_V2 = {"ta": 8, "tb": 8, "ta2": 8, "A": 8, "B": 8, "m": 32, "n": 16,
             "C": 8, "to": 8}


def _op_plan_v2():
    """Cl(3,0) ~= M2(C): A = rho(a)/2, B = rho(b), C = A@B, out = rho^-1(C).

    A layout (interleaved): [A00re, A00im, A01re, A01im, A10re, A10im,
    A11re, A11im]. C layout: [C00re, C01re, C11re, C10re, C01im, C00im,
    C10im, C11im] (gray-code rc order, im block offset by 4, so that all
    combine/output gathers are affine). 80 DVE elems/row vs 192 for v1.
    """
    ops = []
    # A = rho(a)/2 needs a/2: one ACT pass.
    ops.append(("scalar", "smul05", "ta2", [[1, 8]], 0,
                "ta", [[1, 8]], 0, None, None, 0))
    # rho builds: per tensor 4 ops x 2 elems.
    for src, dst in (("ta2", "A"), ("tb", "B")):
        # S1: X00re/im = x0+x3, x4+x7
        ops.append(("vector", "add", dst, [[1, 2]], 0,
                    src, [[4, 2]], 0, src, [[4, 2]], 3))
        # S2: X10re/im = x1+x5, x2+x6
        ops.append(("vector", "add", dst, [[1, 2]], 4,
                    src, [[1, 2]], 1, src, [[1, 2]], 5))
        # D1: X01re = x1-x5, X01im = x6-x2
        ops.append(("vector", "subtract", dst, [[1, 2]], 2,
                    src, [[5, 2]], 1, src, [[-3, 2]], 5))
        # D2: X11re = x0-x3, X11im = x7-x4
        ops.append(("vector", "subtract", dst, [[1, 2]], 6,
                    src, [[7, 2]], 0, src, [[1, 2]], 3))
    # products: one op per complex mult (r,c,t): 4 real products
    # m col = 8*gray(r,c) + 4t + (2sb+sa); [xu, yu, xv, yv] per quad.
    gray = {(0, 0): 0, (0, 1): 1, (1, 1): 2, (1, 0): 3}
    for r in (0, 1):
        for c in (0, 1):
            for tt in (0, 1):
                ops.append(("vector", "mult",
                            "m", [[2, 2], [1, 2]], 8 * gray[(r, c)] + 4 * tt,
                            "A", [[0, 2], [1, 2]], 2 * (2 * r + tt),
                            "B", [[1, 2], [0, 2]], 2 * (2 * tt + c)))
    # t-combine: n[rc, s] = m[rc, t0, s] + m[rc, t1, s]
    ops.append(("vector", "add", "n", [[4, 4], [1, 4]], 0,
                "m", [[8, 4], [1, 4]], 0, "m", [[8, 4], [1, 4]], 4))
    # C re: xu - yv ; C im: yu + xv
    ops.append(("vector", "subtract", "C", [[1, 4]], 0,
                "n", [[4, 4]], 0, "n", [[4, 4]], 3))
    ops.append(("vector", "add", "C", [[2, 2], [-1, 2]], 5,
                "n", [[8, 2], [4, 2]], 1, "n", [[8, 2], [4, 2]], 2))
    # output transform (all halving already folded via ta2)
    ops.append(("vector", "add", "to", [[6, 2], [1, 2]], 0,
                "C", [[4, 2], [1, 2]], 0, "C", [[4, 2], [1, 2]], 2))
    ops.append(("vector", "subtract", "to", [[1, 2]], 3,
                "C", [[5, 2]], 0, "C", [[5, 2]], 2))
    ops.append(("vector", "subtract", "to", [[-3, 2]], 5,
                "C", [[3, 2]], 3, "C", [[3, 2]], 1))
    return ops


_WIDTH_V2C = {"tab": 16, "AB": 16, "m": 32, "n": 16, "C": 8, "tu": 8,
              "to": 8}


def _op_plan_v2c():
    """v2 with a/b interleaved in one DRAM tensor `ab` [rows, 16]:
    one load DMA per tile, rho-builds merged across A and B (4 ops), the
    /2 applied as a final ACT pass on the 8 output cols."""
    ops = []
    # merged rho builds: AB cols 0-7 = A, 8-15 = B (interleaved re/im)
    ops.append(("vector", "add", "AB", [[8, 2], [1, 2]], 0,
                "tab", [[8, 2], [4, 2]], 0, "tab", [[8, 2], [4, 2]], 3))
    ops.append(("vector", "add", "AB", [[8, 2], [1, 2]], 4,
                "tab", [[8, 2], [1, 2]], 1, "tab", [[8, 2], [1, 2]], 5))
    ops.append(("vector", "subtract", "AB", [[8, 2], [1, 2]], 2,
                "tab", [[8, 2], [5, 2]], 1, "tab", [[8, 2], [-3, 2]], 5))
    ops.append(("vector", "subtract", "AB", [[8, 2], [1, 2]], 6,
                "tab", [[8, 2], [7, 2]], 0, "tab", [[8, 2], [1, 2]], 3))
    gray = {(0, 0): 0, (0, 1): 1, (1, 1): 2, (1, 0): 3}
    for r in (0, 1):
        for c in (0, 1):
            for tt in (0, 1):
                ops.append(("vector", "mult",
                            "m", [[2, 2], [1, 2]], 8 * gray[(r, c)] + 4 * tt,
                            "AB", [[0, 2], [1, 2]], 2 * (2 * r + tt),
                            "AB", [[1, 2], [0, 2]], 8 + 2 * (2 * tt + c)))
    ops.append(("vector", "add", "n", [[4, 4], [1, 4]], 0,
                "m", [[8, 4], [1, 4]], 0, "m", [[8, 4], [1, 4]], 4))
    ops.append(("vector", "subtract", "C", [[1, 4]], 0,
                "n", [[4, 4]], 0, "n", [[4, 4]], 3))
    ops.append(("vector", "add", "C", [[2, 2], [-1, 2]], 5,
                "n", [[8, 2], [4, 2]], 1, "n", [[8, 2], [4, 2]], 2))
    ops.append(("vector", "add", "tu", [[6, 2], [1, 2]], 0,
                "C", [[4, 2], [1, 2]], 0, "C", [[4, 2], [1, 2]], 2))
    ops.append(("vector", "subtract", "tu", [[1, 2]], 3,
                "C", [[5, 2]], 0, "C", [[5, 2]], 2))
    ops.append(("vector", "subtract", "tu", [[-3, 2]], 5,
                "C", [[3, 2]], 3, "C", [[3, 2]], 1))
    ops.append(("scalar", "smul05", "to", [[1, 8]], 0,
                "tu", [[1, 8]], 0, None, None, 0))
    return ops


_WIDTH_V2D = {"tab": 16, "AB": 16, "m": 32, "n": 16, "C": 8, "to": 8}


def _op_plan_v2d():
    """v2c with the /2 folded into the host-side `ab` packing (a*0.5 is
    exact in fp32), so the output-combine writes the io tile directly."""
    ops = []
    for op in _op_plan_v2c():
        if op[1] == "smul05":
            continue
        if op[2] == "tu":
            op = op[:2] + ("to",) + op[3:]
        ops.append(op)
    return ops


def _op_plan_v2g():
    """v2c with the t-combine and output-combine stages moved to GPSIMD
    (24 of 80 elems/row), roughly balancing DVE vs GPSIMD time."""
    ops = []
    for op in _op_plan_v2c():
        if op[2] in ("n", "tu") and op[0] == "vector":
            ops.append(("gpsimd",) + op[1:])
        else:
            ops.append(op)
    return ops


def plan_numpy(a, b, plan="v1"):
    """Numpy execution of the exact op plan (for validation in tests)."""
    n = a.shape[0]
    widths = {"v1": _WIDTH, "v2": _WIDTH_V2, "v2c": _WIDTH_V2C,
              "v2g": _WIDTH_V2C, "v2d": _WIDTH_V2D}[plan]
    ab_host = np.concatenate(
        [a * 0.5 if plan == "v2d" else a, b], axis=1).astype(np.float32)
    bufs = {"ta": a.astype(np.float32), "tb": b.astype(np.float32),
            "tab": ab_host}
    if plan == "v1":
        bufs["S"] = np.broadcast_to(_sign_table(), (n, 64))
    for name, w in widths.items():
        if name not in bufs:
            bufs[name] = np.zeros((n, w), np.float32)

    def idx(dims, off):
        out = np.array([off])
        for step, count in dims:
            out = (out[:, None] + step * np.arange(count)[None, :]).ravel()
        return out

    plan_ops = {"v1": _op_plan, "v2": _op_plan_v2, "v2c": _op_plan_v2c,
                "v2g": _op_plan_v2g, "v2d": _op_plan_v2d}[plan]()
    for eng, alu, dst, dd, doff, s0, d0, o0, s1, d1, o1 in plan_ops:
        i_out = idx(dd, doff)
        x0 = bufs[s0][:, idx(d0, o0)]
        if alu == "copy":
            r = x0
        elif alu == "smul05":
            r = x0 * 0.5
        else:
            x1 = bufs[s1][:, idx(d1, o1)]
            r = {"mult": x0 * x1, "add": x0 + x1,
                 "subtract": x0 - x1}[alu]
        bufs[dst][:, i_out] = r
    return bufs["to"]


# ---------------------------------------------------------------------------
# Bass emission
# ---------------------------------------------------------------------------
def _mk_ap(base, dims, off):
    c = base.copy()
    c.ap = mybir.VecI64Pair([list(c.ap[0])] + [list(d) for d in dims])
    c.offset = c.offset + off
    return c


def build_nc(rows_per_core, t=128, repeat=1, ops=None, inplace_z=False,
             io_bufs=3, plan="v1", store_engine="scalar", load_split=False,
             alloc_work=True):
    """Build the SPMD single-core program (same program on all 8 cores).

    repeat > 1 re-processes the same data that many times (used only for
    timing: the wall-clock difference between repeat values isolates pure
    HW kernel time from dispatch overhead).
    """
    tile_rows = P * t
    assert rows_per_core % tile_rows == 0
    ntiles = rows_per_core // tile_rows

    nc = bacc.Bacc("TRN2", target_bir_lowering=False, debug=False,
                   num_devices=N_CORES)
    f32 = mybir.dt.float32
    if plan in ("v2c", "v2g", "v2d"):
        ab_d = nc.dram_tensor("ab", (rows_per_core, 16), f32,
                              kind="ExternalInput")
        abv = ab_d.rearrange("(n p t) c -> n p (t c)", p=P, t=t)
        av = bv = None
    else:
        a_d = nc.dram_tensor("a", (rows_per_core, 8), f32,
                             kind="ExternalInput")
        b_d = nc.dram_tensor("b", (rows_per_core, 8), f32,
                             kind="ExternalInput")
        av = a_d.rearrange("(n p t) c -> n p (t c)", p=P, t=t)
        bv = b_d.rearrange("(n p t) c -> n p (t c)", p=P, t=t)
        abv = None
    s_d = (nc.dram_tensor("sgn", (P, 64), f32, kind="ExternalInput")
           if plan == "v1" else None)
    o_d = nc.dram_tensor("o", (rows_per_core, 8), f32, kind="ExternalOutput")
    ov = o_d.rearrange("(n p t) c -> n p (t c)", p=P, t=t)

    widths = {"v1": _WIDTH, "v2": _WIDTH_V2, "v2c": _WIDTH_V2C,
              "v2g": _WIDTH_V2C, "v2d": _WIDTH_V2D}[plan]
    if ops is None:
        ops = {"v1": _op_plan, "v2": _op_plan_v2, "v2c": _op_plan_v2c,
               "v2g": _op_plan_v2g, "v2d": _op_plan_v2d}[plan]()
    if inplace_z:
        ops = [(e, al, ("za" if d == "z" else d), dd, do,
                ("za" if s0 == "z" else s0), d0, o0,
                ("za" if s1 == "z" else s1), d1, o1)
               for (e, al, d, dd, do, s0, d0, o0, s1, d1, o1) in ops]
    with ExitStack() as ctx:
        tc = ctx.enter_context(tile.TileContext(nc))
        cpool = ctx.enter_context(tc.tile_pool(name="const", bufs=1))
        iopool = ctx.enter_context(tc.tile_pool(name="io", bufs=io_bufs))
        wpool = ctx.enter_context(tc.tile_pool(name="work", bufs=1))

        if plan == "v1":
            s_t = cpool.tile([P, 64], f32)
            nc.sync.dma_start(out=s_t[:], in_=s_d.ap())
        else:
            s_t = None

        for _rep in range(repeat):
          for it in range(ntiles):
            bufs = {"S": s_t}
            if plan in ("v2c", "v2g", "v2d"):
                bufs["tab"] = iopool.tile([P, 16 * t], f32, tag="tab",
                                          name="tab")
                nc.sync.dma_start(out=bufs["tab"][:], in_=abv[it])
            else:
                bufs["ta"] = iopool.tile([P, 8 * t], f32, tag="ta", name="ta")
                bufs["tb"] = iopool.tile([P, 8 * t], f32, tag="tb", name="tb")
                nc.sync.dma_start(out=bufs["ta"][:], in_=av[it])
                (nc.scalar if load_split else nc.sync).dma_start(
                    out=bufs["tb"][:], in_=bv[it])
            if plan == "v1":
                wnames = ("tbp", "za", "t1", "t2") if inplace_z \
                    else ("tbp", "za", "z", "t1", "t2")
            elif plan == "v2":
                wnames = ("ta2", "A", "B", "m", "n", "C")
            elif plan == "v2d":
                wnames = ("AB", "m", "n", "C")
            else:
                wnames = ("AB", "m", "n", "C", "tu")
            if not alloc_work:
                wnames = ()
            for name in wnames:
                bufs[name] = wpool.tile([P, widths[name] * t], f32, tag=name,
                                        name=name)
            bufs["to"] = iopool.tile([P, 8 * t], f32, tag="to", name="to")

            for eng, alu, dst, dd, doff, s0, d0, o0, s1, d1, o1 in ops:
                def full(nm, dims, off):
                    row = [0, t] if nm == "S" else [widths[nm], t]
                    return _mk_ap(bufs[nm][:], [row] + dims, off)
                out_ap = full(dst, dd, doff)
                in0 = full(s0, d0, o0)
                if alu == "copy":
                    nc.scalar.copy(out_ap, in0)
                elif alu == "smul05":
                    nc.scalar.mul(out_ap, in0, 0.5)
                else:
                    in1 = full(s1, d1, o1)
                    e = nc.gpsimd if eng == "gpsimd" else nc.vector
                    fn = {"mult": e.tensor_mul, "add": e.tensor_add,
                          "subtract": e.tensor_sub}[alu]
                    fn(out_ap, in0, in1)

            {"scalar": nc.scalar, "sync": nc.sync,
             "gpsimd": nc.gpsimd}[store_engine].dma_start(
                out=ov[it], in_=bufs["to"][:])

    nc.compile()
    return nc


_NC_CACHE = {}


def _run(a, b, rows_per_core=None, t=256, trace=False, plan="v2d"):
    n = a.shape[0]
    if rows_per_core is None:
        rows_per_core = n // N_CORES
    assert n == rows_per_core * N_CORES
    key = (rows_per_core, t, plan)
    if key not in _NC_CACHE:
        _NC_CACHE[key] = build_nc(rows_per_core, t, plan=plan, io_bufs=4)
    nc = _NC_CACHE[key]
    in_maps = []
    ab = None
    if plan in ("v2c", "v2g"):
        ab = np.concatenate([a, b], axis=1)
    elif plan == "v2d":
        ab = np.concatenate([a * np.float32(0.5), b], axis=1)
    for c in range(N_CORES):
        sl = slice(c * rows_per_core, (c + 1) * rows_per_core)
        if plan in ("v2c", "v2g", "v2d"):
            m = {"ab": np.ascontiguousarray(ab[sl])}
        else:
            m = {"a": np.ascontiguousarray(a[sl]),
                 "b": np.ascontiguousarray(b[sl])}
        if plan == "v1":
            m["sgn"] = np.broadcast_to(_sign_table(), (P, 64)).copy()
        in_maps.append(m)
    res = run_bass_kernel_spmd(nc, in_maps, list(range(N_CORES)), trace=trace)
    out = np.concatenate([res.results[c]["o"] for c in range(N_CORES)], axis=0)
    return out, res


def kernel(a, b, cayley=None):
    a = np.asarray(a, dtype=np.float32)
    b = np.asarray(b, dtype=np.float32)
    assert a.shape == (N_FULL, 8) and b.shape == (N_FULL, 8)
    out, _ = _run(a, b)
    return out


# revision 21
# speedup vs baseline: 1.0412x; 1.0412x over previous
"""Trainium2 Bass kernel: batched Clifford-algebra geometric product, G(3,0,0).

out[n,k] = sum_{i,j} a[n,i] * b[n,j] * cayley[i,j,k],  a/b: [4194304, 8] f32.

Sharding: pure data parallel over 8 NeuronCores (batch axis split into 8
contiguous shards); the cayley structure is compiled into the instruction
stream, so nothing is replicated at runtime.

Per-core algorithm (plan "v2d", the default): instead of expanding all 64
blade products (120+ flops/row), use the algebra isomorphism
Cl(3,0) ~= M2(C).  Per row:

  ab-pack (host): ab = [a/2, b]  (one 2 MB load DMA per tile, /2 is exact)
  rho-build:  A = rho(a)/2, B = rho(b) -- 4 DVE add/sub ops (each entry of
              the 2x2 complex matrices is a +- pair of blade coords)
  product:    C = A @ B -- 8 DVE muls (4 real products per complex mult,
              quad order [xu, yu, xv, yv]) + 1 add (t-contraction) +
              sub/add (re/im combine)
  rho-inverse: out = rho^-1(C) -- 3 DVE add/sub ops

80 DVE elems/row total vs 192 for the direct expansion; rows live in SBUF
partitions (128 rows in flight, 256 rows/partition/tile), all gathers are
small strided access patterns (<= 3 free dims, the ISA limit).  The layouts
of A/B (re/im interleaved), m (gray-code rc order), and C were chosen so
every gather/scatter in the pipeline is affine.

DVE streaming time ~340 us/core; HBM traffic 48 MB/core ~= 150-250 us --
the kernel pipelines tiles so the two mostly overlap (~200-450 us/core
measured, environment is noisy).
"""

import os
import sys

import numpy as np

for _p in ("/opt/trn_rl_repo",):
    if os.path.isdir(_p) and _p not in sys.path:
        sys.path.insert(0, _p)

import concourse.bass as bass  # noqa: E402
import concourse.mybir as mybir  # noqa: E402
from concourse import bacc, tile  # noqa: E402
from concourse.bass_utils import run_bass_kernel_spmd  # noqa: E402
from contextlib import ExitStack  # noqa: E402

N_FULL = 4194304
N_CORES = 8
P = 128  # SBUF partitions


# ---------------------------------------------------------------------------
# Cayley structure (hardcoded G(3,0,0), short-lex blade order)
# ---------------------------------------------------------------------------
def _build_cayley(metric=(1, 1, 1)):
    n = len(metric)
    nb = 1 << n
    bitmaps = sorted(range(nb), key=lambda b: (bin(b).count("1"), b))
    b2i = {bm: i for i, bm in enumerate(bitmaps)}
    C = np.zeros((nb, nb, nb), np.float32)
    for i, bi in enumerate(bitmaps):
        for j, bj in enumerate(bitmaps):
            acc = bi >> 1
            swaps = 0
            while acc:
                swaps += bin(acc & bj).count("1")
                acc >>= 1
            sign = -1.0 if (swaps & 1) else 1.0
            for t in range(n):
                if (bi & bj) & (1 << t):
                    sign *= metric[t]
            if sign:
                C[i, j, b2i[bi ^ bj]] = sign
    return C, np.array(bitmaps)


def _sign_table():
    """S64[8*kb + ib] = sign of (blade ib) * (blade ib^kb) in bitmap order."""
    C, bm = _build_cayley()
    Q = np.argsort(bm)  # bitmap -> short-lex index
    S = np.zeros(64, np.float32)
    for kb in range(8):
        for ib in range(8):
            S[8 * kb + ib] = C[Q[ib], Q[ib ^ kb], Q[kb]]
    return S


# ---------------------------------------------------------------------------
# Op plan: every entry is one engine instruction.
#   (engine, alu, dst, ddims, doff, src0, dims0, off0, src1, dims1, off1)
# dims are [step, count] pairs over the per-row column space of each buffer;
# a leading [row_step=width, t] dim is added at emit time (width 0 for "S").
# Buffer widths (f32 cols per row): ta/tb/tbp/to: 8, za/z: 64, t1: 32, t2: 16.
# ---------------------------------------------------------------------------
_WIDTH = {"ta": 8, "tb": 8, "tbp": 8, "za": 64, "z": 64, "t1": 32, "t2": 16,
          "to": 8, "S": 64}


def _op_plan():
    """Every AP below has at most 3 free dims (+1 row dim added at emit time
    is merged away by the AP optimizer only when contiguous, so ISA allows
    row + 2 more); the ISA TENSOR3D pattern limit is 3 free dims total, so
    each op's dims list must have <= 2 entries beyond what merges with the
    row dim. We keep every dims list <= 2 entries."""
    ops = []
    # b short-lex -> bitmap order (swap cols 3,4); runs on ScalarE.
    ops.append(("scalar", "copy", "tbp", [[5, 2], [1, 3]], 0,
                "tb", [[5, 2], [1, 3]], 0, None, None, 0))
    ops.append(("scalar", "copy", "tbp", [[1, 2]], 3,
                "tb", [[-1, 2]], 4, None, None, 0))
    # sign-fold: za[:, 8k+ib] = S[8k+ib] * a~[:, ib]; a's 3<->4 swap folded in.
    for off in (0, 5):  # ib chunks {0,1,2} and {5,6,7}
        ops.append(("vector", "mult", "za", [[8, 8], [1, 3]], off,
                    "ta", [[0, 8], [1, 3]], off,
                    "S", [[8, 8], [1, 3]], off))
    ops.append(("vector", "mult", "za", [[8, 8], [1, 2]], 3,
                "ta", [[0, 8], [-1, 2]], 4,
                "S", [[8, 8], [1, 2]], 3))
    # products: z[:, 8k+ib] = za[:, 8k+ib] * tbp[:, ib^k].
    # XOR gather = bit-strided dims; merge adjacent bit dims when signs align.
    for k in range(8):
        s2 = -4 if k & 4 else 4
        s1 = -2 if k & 2 else 2
        s0 = -1 if k & 1 else 1
        if (s1 > 0) == (s0 > 0):      # merge (b1,b0) -> step s0 x4
            ops.append(("vector", "mult", "z", [[4, 2], [1, 4]], 8 * k,
                        "za", [[4, 2], [1, 4]], 8 * k,
                        "tbp", [[s2, 2], [s0, 4]], k))
        elif (s2 > 0) == (s1 > 0):    # merge (b2,b1) -> step s1 x4
            ops.append(("vector", "mult", "z", [[2, 4], [1, 2]], 8 * k,
                        "za", [[2, 4], [1, 2]], 8 * k,
                        "tbp", [[s1, 4], [s0, 2]], k))
        else:                          # k in {2,5}: split over b2
            for i2 in (0, 1):
                ops.append(("vector", "mult",
                            "z", [[2, 2], [1, 2]], 8 * k + 4 * i2,
                            "za", [[2, 2], [1, 2]], 8 * k + 4 * i2,
                            "tbp", [[s1, 2], [s0, 2]], k ^ (4 * i2)))
    # tree adds over ib (4 inner cols at a time)
    ops.append(("vector", "add", "t1", [[4, 8], [1, 4]], 0,
                "z", [[8, 8], [1, 4]], 0, "z", [[8, 8], [1, 4]], 4))
    ops.append(("vector", "add", "t2", [[2, 8], [1, 2]], 0,
                "t1", [[4, 8], [1, 2]], 0, "t1", [[4, 8], [1, 2]], 2))
    # final add, bitmap -> short-lex fold on the output
    ops.append(("vector", "add", "to", [[5, 2], [1, 3]], 0,
                "t2", [[10, 2], [2, 3]], 0, "t2", [[10, 2], [2, 3]], 1))
    ops.append(("vector", "add", "to", [[-1, 2]], 4,
                "t2", [[2, 2]], 6, "t2", [[2, 2]], 7))
    return ops


_WIDTH_V2 = {"ta": 8, "tb": 8, "ta2": 8, "A": 8, "B": 8, "m": 32, "n": 16,
             "C": 8, "to": 8}


def _op_plan_v2():
    """Cl(3,0) ~= M2(C): A = rho(a)/2, B = rho(b), C = A@B, out = rho^-1(C).

    A layout (interleaved): [A00re, A00im, A01re, A01im, A10re, A10im,
    A11re, A11im]. C layout: [C00re, C01re, C11re, C10re, C01im, C00im,
    C10im, C11im] (gray-code rc order, im block offset by 4, so that all
    combine/output gathers are affine). 80 DVE elems/row vs 192 for v1.
    """
    ops = []
    # A = rho(a)/2 needs a/2: one ACT pass.
    ops.append(("scalar", "smul05", "ta2", [[1, 8]], 0,
                "ta", [[1, 8]], 0, None, None, 0))
    # rho builds: per tensor 4 ops x 2 elems.
    for src, dst in (("ta2", "A"), ("tb", "B")):
        # S1: X00re/im = x0+x3, x4+x7
        ops.append(("vector", "add", dst, [[1, 2]], 0,
                    src, [[4, 2]], 0, src, [[4, 2]], 3))
        # S2: X10re/im = x1+x5, x2+x6
        ops.append(("vector", "add", dst, [[1, 2]], 4,
                    src, [[1, 2]], 1, src, [[1, 2]], 5))
        # D1: X01re = x1-x5, X01im = x6-x2
        ops.append(("vector", "subtract", dst, [[1, 2]], 2,
                    src, [[5, 2]], 1, src, [[-3, 2]], 5))
        # D2: X11re = x0-x3, X11im = x7-x4
        ops.append(("vector", "subtract", dst, [[1, 2]], 6,
                    src, [[7, 2]], 0, src, [[1, 2]], 3))
    # products: one op per complex mult (r,c,t): 4 real products
    # m col = 8*gray(r,c) + 4t + (2sb+sa); [xu, yu, xv, yv] per quad.
    gray = {(0, 0): 0, (0, 1): 1, (1, 1): 2, (1, 0): 3}
    for r in (0, 1):
        for c in (0, 1):
            for tt in (0, 1):
                ops.append(("vector", "mult",
                            "m", [[2, 2], [1, 2]], 8 * gray[(r, c)] + 4 * tt,
                            "A", [[0, 2], [1, 2]], 2 * (2 * r + tt),
                            "B", [[1, 2], [0, 2]], 2 * (2 * tt + c)))
    # t-combine: n[rc, s] = m[rc, t0, s] + m[rc, t1, s]
    ops.append(("vector", "add", "n", [[4, 4], [1, 4]], 0,
                "m", [[8, 4], [1, 4]], 0, "m", [[8, 4], [1, 4]], 4))
    # C re: xu - yv ; C im: yu + xv
    ops.append(("vector", "subtract", "C", [[1, 4]], 0,
                "n", [[4, 4]], 0, "n", [[4, 4]], 3))
    ops.append(("vector", "add", "C", [[2, 2], [-1, 2]], 5,
                "n", [[8, 2], [4, 2]], 1, "n", [[8, 2], [4, 2]], 2))
    # output transform (all halving already folded via ta2)
    ops.append(("vector", "add", "to", [[6, 2], [1, 2]], 0,
                "C", [[4, 2], [1, 2]], 0, "C", [[4, 2], [1, 2]], 2))
    ops.append(("vector", "subtract", "to", [[1, 2]], 3,
                "C", [[5, 2]], 0, "C", [[5, 2]], 2))
    ops.append(("vector", "subtract", "to", [[-3, 2]], 5,
                "C", [[3, 2]], 3, "C", [[3, 2]], 1))
    return ops


_WIDTH_V2C = {"tab": 16, "AB": 16, "m": 32, "n": 16, "C": 8, "tu": 8,
              "to": 8}


def _op_plan_v2c():
    """v2 with a/b interleaved in one DRAM tensor `ab` [rows, 16]:
    one load DMA per tile, rho-builds merged across A and B (4 ops), the
    /2 applied as a final ACT pass on the 8 output cols."""
    ops = []
    # merged rho builds: AB cols 0-7 = A, 8-15 = B (interleaved re/im)
    ops.append(("vector", "add", "AB", [[8, 2], [1, 2]], 0,
                "tab", [[8, 2], [4, 2]], 0, "tab", [[8, 2], [4, 2]], 3))
    ops.append(("vector", "add", "AB", [[8, 2], [1, 2]], 4,
                "tab", [[8, 2], [1, 2]], 1, "tab", [[8, 2], [1, 2]], 5))
    ops.append(("vector", "subtract", "AB", [[8, 2], [1, 2]], 2,
                "tab", [[8, 2], [5, 2]], 1, "tab", [[8, 2], [-3, 2]], 5))
    ops.append(("vector", "subtract", "AB", [[8, 2], [1, 2]], 6,
                "tab", [[8, 2], [7, 2]], 0, "tab", [[8, 2], [1, 2]], 3))
    gray = {(0, 0): 0, (0, 1): 1, (1, 1): 2, (1, 0): 3}
    for r in (0, 1):
        for c in (0, 1):
            for tt in (0, 1):
                ops.append(("vector", "mult",
                            "m", [[2, 2], [1, 2]], 8 * gray[(r, c)] + 4 * tt,
                            "AB", [[0, 2], [1, 2]], 2 * (2 * r + tt),
                            "AB", [[1, 2], [0, 2]], 8 + 2 * (2 * tt + c)))
    ops.append(("vector", "add", "n", [[4, 4], [1, 4]], 0,
                "m", [[8, 4], [1, 4]], 0, "m", [[8, 4], [1, 4]], 4))
    ops.append(("vector", "subtract", "C", [[1, 4]], 0,
                "n", [[4, 4]], 0, "n", [[4, 4]], 3))
    ops.append(("vector", "add", "C", [[2, 2], [-1, 2]], 5,
                "n", [[8, 2], [4, 2]], 1, "n", [[8, 2], [4, 2]], 2))
    ops.append(("vector", "add", "tu", [[6, 2], [1, 2]], 0,
                "C", [[4, 2], [1, 2]], 0, "C", [[4, 2], [1, 2]], 2))
    ops.append(("vector", "subtract", "tu", [[1, 2]], 3,
                "C", [[5, 2]], 0, "C", [[5, 2]], 2))
    ops.append(("vector", "subtract", "tu", [[-3, 2]], 5,
                "C", [[3, 2]], 3, "C", [[3, 2]], 1))
    ops.append(("scalar", "smul05", "to", [[1, 8]], 0,
                "tu", [[1, 8]], 0, None, None, 0))
    return ops


_WIDTH_V2D = {"tab": 16, "AB": 16, "m": 32, "n": 16, "C": 8, "to": 8}


def _op_plan_v2d():
    """v2c with the /2 folded into the host-side `ab` packing (a*0.5 is
    exact in fp32), so the output-combine writes the io tile directly."""
    ops = []
    for op in _op_plan_v2c():
        if op[1] == "smul05":
            continue
        if op[2] == "tu":
            op = op[:2] + ("to",) + op[3:]
        ops.append(op)
    return ops


def _op_plan_v2g():
    """v2c with the t-combine and output-combine stages moved to GPSIMD
    (24 of 80 elems/row), roughly balancing DVE vs GPSIMD time."""
    ops = []
    for op in _op_plan_v2c():
        if op[2] in ("n", "tu") and op[0] == "vector":
            ops.append(("gpsimd",) + op[1:])
        else:
            ops.append(op)
    return ops


def plan_numpy(a, b, plan="v1"):
    """Numpy execution of the exact op plan (for validation in tests)."""
    n = a.shape[0]
    widths = {"v1": _WIDTH, "v2": _WIDTH_V2, "v2c": _WIDTH_V2C,
              "v2g": _WIDTH_V2C, "v2d": _WIDTH_V2D}[plan]
    ab_host = np.concatenate(
        [a * 0.5 if plan == "v2d" else a, b], axis=1).astype(np.float32)
    bufs = {"ta": a.astype(np.float32), "tb": b.astype(np.float32),
            "tab": ab_host}
    if plan == "v1":
        bufs["S"] = np.broadcast_to(_sign_table(), (n, 64))
    for name, w in widths.items():
        if name not in bufs:
            bufs[name] = np.zeros((n, w), np.float32)

    def idx(dims, off):
        out = np.array([off])
        for step, count in dims:
            out = (out[:, None] + step * np.arange(count)[None, :]).ravel()
        return out

    plan_ops = {"v1": _op_plan, "v2": _op_plan_v2, "v2c": _op_plan_v2c,
                "v2g": _op_plan_v2g, "v2d": _op_plan_v2d}[plan]()
    for eng, alu, dst, dd, doff, s0, d0, o0, s1, d1, o1 in plan_ops:
        i_out = idx(dd, doff)
        x0 = bufs[s0][:, idx(d0, o0)]
        if alu == "copy":
            r = x0
        elif alu == "smul05":
            r = x0 * 0.5
        else:
            x1 = bufs[s1][:, idx(d1, o1)]
            r = {"mult": x0 * x1, "add": x0 + x1,
                 "subtract": x0 - x1}[alu]
        bufs[dst][:, i_out] = r
    return bufs["to"]


# ---------------------------------------------------------------------------
# Bass emission
# ---------------------------------------------------------------------------
def _mk_ap(base, dims, off):
    c = base.copy()
    c.ap = mybir.VecI64Pair([list(c.ap[0])] + [list(d) for d in dims])
    c.offset = c.offset + off
    return c


def build_nc(rows_per_core, t=128, repeat=1, ops=None, inplace_z=False,
             io_bufs=3, plan="v1", store_engine="scalar", load_split=False,
             alloc_work=True, nodep=False):
    """Build the SPMD single-core program (same program on all 8 cores).

    repeat > 1 re-processes the same data that many times (used only for
    timing: the wall-clock difference between repeat values isolates pure
    HW kernel time from dispatch overhead).
    """
    tile_rows = P * t
    assert rows_per_core % tile_rows == 0
    ntiles = rows_per_core // tile_rows

    nc = bacc.Bacc("TRN2", target_bir_lowering=False, debug=False,
                   num_devices=N_CORES)
    f32 = mybir.dt.float32
    if plan in ("v2c", "v2g", "v2d"):
        ab_d = nc.dram_tensor("ab", (rows_per_core, 16), f32,
                              kind="ExternalInput")
        abv = ab_d.rearrange("(n p t) c -> n p (t c)", p=P, t=t)
        av = bv = None
    else:
        a_d = nc.dram_tensor("a", (rows_per_core, 8), f32,
                             kind="ExternalInput")
        b_d = nc.dram_tensor("b", (rows_per_core, 8), f32,
                             kind="ExternalInput")
        av = a_d.rearrange("(n p t) c -> n p (t c)", p=P, t=t)
        bv = b_d.rearrange("(n p t) c -> n p (t c)", p=P, t=t)
        abv = None
    s_d = (nc.dram_tensor("sgn", (P, 64), f32, kind="ExternalInput")
           if plan == "v1" else None)
    o_d = nc.dram_tensor("o", (rows_per_core, 8), f32, kind="ExternalOutput")
    ov = o_d.rearrange("(n p t) c -> n p (t c)", p=P, t=t)

    widths = {"v1": _WIDTH, "v2": _WIDTH_V2, "v2c": _WIDTH_V2C,
              "v2g": _WIDTH_V2C, "v2d": _WIDTH_V2D}[plan]
    if ops is None:
        ops = {"v1": _op_plan, "v2": _op_plan_v2, "v2c": _op_plan_v2c,
               "v2g": _op_plan_v2g, "v2d": _op_plan_v2d}[plan]()
    if inplace_z:
        ops = [(e, al, ("za" if d == "z" else d), dd, do,
                ("za" if s0 == "z" else s0), d0, o0,
                ("za" if s1 == "z" else s1), d1, o1)
               for (e, al, d, dd, do, s0, d0, o0, s1, d1, o1) in ops]
    with ExitStack() as ctx:
        tc = ctx.enter_context(tile.TileContext(nc))
        cpool = ctx.enter_context(tc.tile_pool(name="const", bufs=1))
        iopool = ctx.enter_context(tc.tile_pool(name="io", bufs=io_bufs))
        wpool = ctx.enter_context(tc.tile_pool(name="work", bufs=1))

        if plan == "v1":
            s_t = cpool.tile([P, 64], f32)
            nc.sync.dma_start(out=s_t[:], in_=s_d.ap())
        else:
            s_t = None
        tab0 = None
        if nodep:
            # decoupled-compute probe: DVE chain reads this never-loaded
            # constant tile instead of the DMA-landed tab
            tab0 = cpool.tile([P, 16 * t], f32)
            nc.gpsimd.memset(tab0[:], 0.0)

        for _rep in range(repeat):
          for it in range(ntiles):
            bufs = {"S": s_t}
            if plan in ("v2c", "v2g", "v2d"):
                bufs["tab"] = iopool.tile([P, 16 * t], f32, tag="tab",
                                          name="tab")
                nc.sync.dma_start(out=bufs["tab"][:], in_=abv[it])
                if nodep:
                    bufs["tab"] = tab0
            else:
                bufs["ta"] = iopool.tile([P, 8 * t], f32, tag="ta", name="ta")
                bufs["tb"] = iopool.tile([P, 8 * t], f32, tag="tb", name="tb")
                nc.sync.dma_start(out=bufs["ta"][:], in_=av[it])
                (nc.scalar if load_split else nc.sync).dma_start(
                    out=bufs["tb"][:], in_=bv[it])
            if plan == "v1":
                wnames = ("tbp", "za", "t1", "t2") if inplace_z \
                    else ("tbp", "za", "z", "t1", "t2")
            elif plan == "v2":
                wnames = ("ta2", "A", "B", "m", "n", "C")
            elif plan == "v2d":
                wnames = ("AB", "m", "n", "C")
            else:
                wnames = ("AB", "m", "n", "C", "tu")
            if not alloc_work:
                wnames = ()
            for name in wnames:
                bufs[name] = wpool.tile([P, widths[name] * t], f32, tag=name,
                                        name=name)
            bufs["to"] = iopool.tile([P, 8 * t], f32, tag="to", name="to")

            for eng, alu, dst, dd, doff, s0, d0, o0, s1, d1, o1 in ops:
                def full(nm, dims, off):
                    row = [0, t] if nm == "S" else [widths[nm], t]
                    return _mk_ap(bufs[nm][:], [row] + dims, off)
                out_ap = full(dst, dd, doff)
                in0 = full(s0, d0, o0)
                if alu == "copy":
                    nc.scalar.copy(out_ap, in0)
                elif alu == "smul05":
                    nc.scalar.mul(out_ap, in0, 0.5)
                else:
                    in1 = full(s1, d1, o1)
                    e = nc.gpsimd if eng == "gpsimd" else nc.vector
                    fn = {"mult": e.tensor_mul, "add": e.tensor_add,
                          "subtract": e.tensor_sub}[alu]
                    fn(out_ap, in0, in1)

            {"scalar": nc.scalar, "sync": nc.sync,
             "gpsimd": nc.gpsimd}[store_engine].dma_start(
                out=ov[it], in_=bufs["to"][:])

    nc.compile()
    return nc


_NC_CACHE = {}


def _run(a, b, rows_per_core=None, t=256, trace=False, plan="v2d"):
    n = a.shape[0]
    if rows_per_core is None:
        rows_per_core = n // N_CORES
    assert n == rows_per_core * N_CORES
    key = (rows_per_core, t, plan)
    if key not in _NC_CACHE:
        _NC_CACHE[key] = build_nc(rows_per_core, t, plan=plan, io_bufs=4)
    nc = _NC_CACHE[key]
    in_maps = []
    ab = None
    if plan in ("v2c", "v2g"):
        ab = np.concatenate([a, b], axis=1)
    elif plan == "v2d":
        ab = np.concatenate([a * np.float32(0.5), b], axis=1)
    for c in range(N_CORES):
        sl = slice(c * rows_per_core, (c + 1) * rows_per_core)
        if plan in ("v2c", "v2g", "v2d"):
            m = {"ab": np.ascontiguousarray(ab[sl])}
        else:
            m = {"a": np.ascontiguousarray(a[sl]),
                 "b": np.ascontiguousarray(b[sl])}
        if plan == "v1":
            m["sgn"] = np.broadcast_to(_sign_table(), (P, 64)).copy()
        in_maps.append(m)
    res = run_bass_kernel_spmd(nc, in_maps, list(range(N_CORES)), trace=trace)
    out = np.concatenate([res.results[c]["o"] for c in range(N_CORES)], axis=0)
    return out, res


def kernel(a, b, cayley=None):
    a = np.asarray(a, dtype=np.float32)
    b = np.asarray(b, dtype=np.float32)
    assert a.shape == (N_FULL, 8) and b.shape == (N_FULL, 8)
    out, _ = _run(a, b)
    return out
